# revision 26
# baseline (speedup 1.0000x reference)
# Trainium2 Bass kernel for nn_LogitsNew (dense_mlp).
#
#   u = gelu(x @ W_proj + b_proj)                       [B, D]
#   logits = (u @ W_u)[:, None, :] + ee @ W_e           [B, N, C]
#
# Sharding: data-parallel over batch B across 8 cores (4 batches/core).
#
# v5 design (93.7us baseline -> 65.8 -> this):
#   - fp16 end to end (fp32 PSUM), host-side packing: ee pre-transposed
#     into PE-stationary layout, x pre-transposed + b_proj as a
#     per-partition column.
#   - ALL inputs on one DMA ring (ACT) in PE-consumption order; the 16
#     DMA queues round-robin between rings per descriptor, so bulk loads
#     on a second ring would starve the critical chunks. Stores on SP.
#   - z and y computed TRANSPOSED (W-chunk stationary, 4-wide moving):
#     gelu(zT)+bias-col gives uT directly; yT -> y16 via PE transposes.
#   - y broadcast across partitions via K=1 matmuls (ones stationary,
#     y16 row moving) — the gpsimd partition_broadcast path costs ~2us
#     per op and was the tail bottleneck in v4.
#   - 12 warmup matmuls lift the HAM clock gate (1.2->2.4GHz) during the
#     initial DMA wait.
#   - drains m0-m4 on scalar + vector adds; m5-m7 drain fused with the
#     +y add on vector (PSUM read). 3 grouped stores, fp16 out, host
#     unpacks the [p, mt, c] device layout.

import sys

if "/opt/trn_rl_repo" not in sys.path:
    sys.path.insert(0, "/opt/trn_rl_repo")

import numpy as np

import concourse.bass as bass
import concourse.mybir as mybir
import concourse.tile as tile
from concourse import bacc
from concourse.bass_utils import run_bass_kernel_spmd
from concourse.masks import make_identity

P = 128
B, N, D, C = 32, 256, 1024, 1024
NCORES = 8
BPC = B // NCORES          # batches per core
KT = D // P                # 8 k-tiles over the contraction dim
FD = 512                   # matmul moving free dim (one PSUM bank of fp32)
NT = N // P                # 2 n-tiles per batch
MT = BPC * NT              # 8 m-tiles per core
XBW = KT * BPC + KT + BPC * P  # xb: xT (32) + b column (8) + y-bcast selectors (512)

SEL0 = KT * BPC + KT       # column offset of the selector blocks in xb
F32 = mybir.dt.float32
F16 = mybir.dt.float16
GELU = mybir.ActivationFunctionType.Gelu

_CACHE = {}


def _build():
    if "nc" in _CACHE:
        return _CACHE["nc"]

    nc = bacc.Bacc("TRN2", target_bir_lowering=False, debug=False, num_devices=NCORES)

    # host-packed inputs (all fp16):
    #   ee_t[p, m, k, f] = ee[b, nh*128+f, k*128+p], m = b*NT+nh
    #   we_t/wu_t/wp_t[p, k, c] = W[k*128+p, c]
    #   xb[p, k*BPC+b] = x[b, k*128+p]; xb[p, 32+k] = b_proj[k*128+p]
    ee_t = nc.dram_tensor("ee_t", [P, MT, KT, P], F16, kind="ExternalInput").ap()
    we_t = nc.dram_tensor("we_t", [P, KT, C], F16, kind="ExternalInput").ap()
    wu_t = nc.dram_tensor("wu_t", [P, KT, C], F16, kind="ExternalInput").ap()
    wp_t = nc.dram_tensor("wp_t", [P, KT, C], F16, kind="ExternalInput").ap()
    xb = nc.dram_tensor("xb", [P, XBW], F16, kind="ExternalInput").ap()
    out = nc.dram_tensor("out_t", [P, MT, C], F16, kind="ExternalOutput").ap()

    with tile.TileContext(nc) as tc:
        with (
            tc.tile_pool(name="const", bufs=1) as cpool,
            tc.tile_pool(name="weights", bufs=1) as wpool,
            tc.tile_pool(name="outs", bufs=1) as outpool,
            tc.tile_pool(name="warm_ps", bufs=1, space="PSUM") as warm_ps,
            tc.tile_pool(name="zy_ps", bufs=2, space="PSUM") as zy_ps,
            tc.tile_pool(name="mm_ps", bufs=4, space="PSUM") as mm_ps,
        ):
            xb_sb = cpool.tile([P, XBW], F16)
            we16 = wpool.tile([P, KT, C], F16)
            wp16 = wpool.tile([P, KT, C], F16)
            wu16 = wpool.tile([P, KT, C], F16)
            ee_sb = cpool.tile([P, MT, KT, P], F16)
            o_all = outpool.tile([P, MT, C], F16)

            # ---- ALL inputs on the ACT ring in PE-consumption order ----
            nc.scalar.dma_start(xb_sb, xb)
            nc.scalar.dma_start(ee_sb[:, 0:1], ee_t[:, 0:1])
            nc.scalar.dma_start(ee_sb[:, 1:2], ee_t[:, 1:2])
            for j in range(4):
                nc.scalar.dma_start(we16[:, 2 * j : 2 * j + 2], we_t[:, 2 * j : 2 * j + 2])
            nc.scalar.dma_start(ee_sb[:, 2:4], ee_t[:, 2:4])
            nc.scalar.dma_start(wp16, wp_t)
            nc.scalar.dma_start(wu16, wu_t)
            nc.scalar.dma_start(ee_sb[:, 4:8], ee_t[:, 4:8])

            # preload the GELU activation table (2.6us ACT_TABLE_LOAD) while
            # the input DMAs stream, off the scalar queue's critical path
            gelu_warm = cpool.tile([1, 2], F32)
            nc.scalar.activation(gelu_warm, xb_sb[0:1, 0:2], GELU)

            # constants: ones row (warmup), one-hot selectors (y broadcast),
            # identity (y transposes)
            ones = cpool.tile([P, P + FD], F16)
            nc.gpsimd.memset(ones, 1.0)
            ident_f = cpool.tile([P, P], F32)
            make_identity(nc, ident_f)
            ident = cpool.tile([P, P], F16)
            nc.scalar.copy(ident, ident_f)

            # ---- PE warmup: junk matmuls bridging the initial DMA wait ----
            for _ in range(13):
                wp_ps = warm_ps.tile([P, FD], F32, tag="warm")
                nc.tensor.matmul(wp_ps, ones[:1, :P], ones[:1, P:], start=True, stop=True)

            ybc = cpool.tile([P, 2, C], F32)
            y16 = cpool.tile([BPC, C], F16)

            def mtile_kloop(mt, fuse_y):
                mps = [
                    mm_ps.tile([P, FD], F32, tag="mm", name=f"mm_{mt}_{ch}")
                    for ch in range(2)
                ]
                for k in range(KT):
                    for ch in range(2):
                        nc.tensor.matmul(
                            mps[ch],
                            ee_sb[:, mt, k, :],
                            we16[:, k, ch * FD : (ch + 1) * FD],
                            start=(k == 0),
                            stop=(k == KT - 1) and not fuse_y,
                        )
                return mps

            def mtile_finish(mt, fuse_y, mps):
                # fuse_y: accumulate the broadcast +y directly in PSUM via a
                # final K=4 selector matmul, then drain with a plain copy.
                b = mt // NT
                for ch in range(2):
                    cs = slice(ch * FD, (ch + 1) * FD)
                    if fuse_y:
                        nc.tensor.matmul(
                            mps[ch],
                            xb_sb[:BPC, SEL0 + b * P : SEL0 + (b + 1) * P],
                            y16[:, cs],
                            start=False,
                            stop=True,
                        )
                    nc.scalar.copy(o_all[:, mt, cs], mps[ch])

            def main_mtile(mt, fuse_y):
                mtile_finish(mt, fuse_y, mtile_kloop(mt, fuse_y))

            main_mtile(0, False)
            main_mtile(1, False)
            main_mtile(2, False)

            main_mtile(3, False)
            # ---- zT and m4 fill the window while W_u streams ----
            # ---- zT = (x @ W_proj).T ; uT = gelu(zT + b) ----
            ztp = zy_ps.tile([P, KT * BPC], F32, tag="zy", name="zt")
            for kc in range(KT):
                for kd in range(KT):
                    nc.tensor.matmul(
                        ztp[:, kc * BPC : (kc + 1) * BPC],
                        wp16[:, kd, kc * P : (kc + 1) * P],
                        xb_sb[:, kd * BPC : (kd + 1) * BPC],
                        start=(kd == 0),
                        stop=(kd == KT - 1),
                    )
            uT = cpool.tile([P, KT, BPC], F16)
            for kc in range(KT):
                nc.scalar.activation(
                    uT[:, kc, :],
                    ztp[:, kc * BPC : (kc + 1) * BPC],
                    GELU,
                    bias=xb_sb[:, KT * BPC + kc : KT * BPC + kc + 1],
                )

            # ---- yT = (u @ W_u).T (W_u-chunk stationary, 4-wide moving) ----
            ytp = zy_ps.tile([P, KT * BPC], F32, tag="zy", name="yt")
            for kc in range(KT):
                for kd in range(KT):
                    nc.tensor.matmul(
                        ytp[:, kc * BPC : (kc + 1) * BPC],
                        wu16[:, kd, kc * P : (kc + 1) * P],
                        uT[:, kd, :],
                        start=(kd == 0),
                        stop=(kd == KT - 1),
                    )
            yT16 = cpool.tile([P, KT, BPC], F16)
            nc.scalar.copy(yT16, ytp)

            # ---- y16 = yT.T via PE transposes; ybc via selector matmuls ----
            for kc in range(KT):
                tp = zy_ps.tile([BPC, P], F16, tag="zy", name=f"tp_{kc}")
                nc.tensor.transpose(tp, yT16[:, kc, :], ident)
                nc.scalar.copy(y16[:, kc * P : (kc + 1) * P], tp)
            main_mtile(4, True)

            for b2 in range(2):
                for ch in range(2):
                    cs = slice(ch * FD, (ch + 1) * FD)
                    bc = zy_ps.tile([P, FD], F32, tag="zy", name=f"bc_{b2}_{ch}")
                    nc.tensor.matmul(
                        bc,
                        xb_sb[:BPC, SEL0 + b2 * P : SEL0 + (b2 + 1) * P],
                        y16[:, cs],
                        start=True,
                        stop=True,
                    )
                    nc.vector.tensor_copy(ybc[:, b2, cs], bc)

            # ---- +y adds for the early tiles (vector, hidden under PE) ----
            for mt in range(4):
                b = mt // NT
                nc.vector.tensor_add(o_all[:, mt, :], o_all[:, mt, :], ybc[:, b, :])

            main_mtile(5, True)
            main_mtile(6, True)
            main_mtile(7, True)

            # ---- grouped stores on the SP ring ----
            nc.sync.dma_start(out[:, 0:4], o_all[:, 0:4])
            nc.sync.dma_start(out[:, 4:7], o_all[:, 4:7])
            nc.sync.dma_start(out[:, 7:8], o_all[:, 7:8])

    nc.compile()
    _CACHE["nc"] = nc
    return nc


def _pack(inputs):
    """Host-side dtype conversion + layout packing (no arithmetic)."""
    x = np.asarray(inputs["encoded_utterance"], np.float32)
    ee = np.asarray(inputs["element_embeddings"], np.float32)
    w = np.asarray(inputs["weight_matrix"], np.float32)
    wp = np.asarray(inputs["W_proj"], np.float32)
    b = np.asarray(inputs["b_proj"], np.float32).reshape(D)

    wu_t = np.ascontiguousarray(
        w[:D].reshape(KT, P, C).transpose(1, 0, 2).astype(np.float16)
    )
    we_t = np.ascontiguousarray(
        w[D:].reshape(KT, P, C).transpose(1, 0, 2).astype(np.float16)
    )
    wp_t = np.ascontiguousarray(
        wp.reshape(KT, P, C).transpose(1, 0, 2).astype(np.float16)
    )
    bcol = b.reshape(KT, P).T.astype(np.float16)  # [p, k]

    ee16 = ee.astype(np.float16)
    x16 = x.astype(np.float16)
    in_maps = []
    for i in range(NCORES):
        bs = slice(i * BPC, (i + 1) * BPC)
        ee_ti = np.ascontiguousarray(
            ee16[bs].reshape(MT, P, KT, P).transpose(3, 0, 2, 1)
        )
        xbm = np.zeros((P, XBW), np.float16)
        xbm[:, : KT * BPC] = (
            x16[bs].T.reshape(KT, P, BPC).transpose(1, 0, 2).reshape(P, KT * BPC)
        )
        xbm[:, KT * BPC : SEL0] = bcol
        for b2 in range(BPC):
            xbm[b2, SEL0 + b2 * P : SEL0 + (b2 + 1) * P] = 1.0
        in_maps.append(
            {"ee_t": ee_ti, "we_t": we_t, "wu_t": wu_t, "wp_t": wp_t, "xb": xbm}
        )
    return in_maps


def run(inputs, trace=False, **kwargs):
    nc = _build()
    in_maps = _pack(inputs)
    res = run_bass_kernel_spmd(
        nc, in_maps, core_ids=list(range(NCORES)), trace=trace, **kwargs
    )
    # out_t[p, m, c] -> logits[b, nh*128+p, c]
    outs = []
    for r in res.results:
        o = r["out_t"].astype(np.float32)  # [P, MT, C]
        outs.append(o.transpose(1, 0, 2).reshape(BPC, N, C))
    full = np.concatenate(outs, axis=0)
    return full, res


def kernel(**inputs) -> np.ndarray:
    return run(inputs, trace=False)[0]
